# revision 1
# baseline (speedup 1.0000x reference)
"""nn_InvResMLP kernel — data-parallel over batch across 8 NeuronCores.

Sharding: batch dim (8) maps one point cloud per core; ball query, grouping,
and the conv/BN/MLP stack are per-cloud so no cross-device communication is
needed. Weights are replicated. Falls back to a numpy implementation if
device execution is unavailable.
"""
import numpy as np

RADIUS = 0.15
K = 32
EPS = 1e-5
B, N, C = 8, 4096, 128


def _kernel_numpy(pos, x, W1, g1, b1, m1, v1, W2, g2, b2, m2, v2,
                  W3, g3, b3, m3, v3):
    def bn(y, g, b, m, v):
        return (y - m) * (g / np.sqrt(v + EPS)) + b

    pos = np.asarray(pos, np.float32)
    x = np.asarray(x, np.float32)
    outs = []
    for bi in range(pos.shape[0]):
        p = pos[bi]
        sq = np.sum(p * p, axis=-1)
        d2 = sq[:, None] + sq[None, :] - 2.0 * (p @ p.T)
        ar = np.arange(N, dtype=np.int64)
        keys = np.where(d2 < RADIUS * RADIUS, ar[None, :], ar[None, :] + N)
        topk = np.sort(keys, axis=1)[:, :K]
        valid = topk < N
        idx = np.where(valid, topk, topk - N)
        idx = np.where(valid, idx, idx[:, 0:1])
        gp = p[idx]
        gf = x[bi][idx]
        rel = (gp - p[:, None, :]) / RADIUS
        res = np.concatenate([rel, gf], axis=-1)
        y = res @ W1.T
        y = np.maximum(bn(y, g1, b1, m1, v1), 0.0)
        f = y.max(axis=1)
        h = np.maximum(bn(f @ W2.T, g2, b2, m2, v2), 0.0)
        h = bn(h @ W3.T, g3, b3, m3, v3)
        outs.append(np.maximum(x[bi] + h, 0.0))
    return pos, np.stack(outs)


def _make_device_fn():
    import jax
    import jax.numpy as jnp

    def bn(y, g, b, m, v):
        return (y - m) * (g / jnp.sqrt(v + EPS)) + b

    def per_cloud(pos, x, W1, g1, b1, m1, v1, W2, g2, b2, m2, v2,
                  W3, g3, b3, m3, v3):
        # ball query (first-K-in-index-order within RADIUS), self-grouping
        sq = jnp.sum(pos * pos, axis=-1)
        d2 = sq[:, None] + sq[None, :] - 2.0 * (pos @ pos.T)
        ar = jnp.arange(N, dtype=jnp.int32)
        keys = jnp.where(d2 < RADIUS * RADIUS, ar[None, :], ar[None, :] + N)
        topk = -jax.lax.top_k(-keys, K)[0]
        valid = topk < N
        idx = jnp.where(valid, topk, topk - N)
        idx = jnp.where(valid, idx, idx[:, :1])
        gp = pos[idx]                                  # (n, k, 3)
        gf = x[idx]                                    # (n, k, c)
        rel = (gp - pos[:, None, :]) / RADIUS
        res = jnp.concatenate([rel, gf], axis=-1)      # (n, k, 3+c)
        y = jnp.einsum('nki,oi->nko', res, W1)
        y = jax.nn.relu(bn(y, g1, b1, m1, v1))
        f = jnp.max(y, axis=1)                         # (n, c)
        h = jnp.einsum('nc,oc->no', f, W2)
        h = jax.nn.relu(bn(h, g2, b2, m2, v2))
        h = jnp.einsum('nc,oc->no', h, W3)
        h = bn(h, g3, b3, m3, v3)
        return jax.nn.relu(x + h)

    return per_cloud


def kernel(pos, x, W1, g1, b1, m1, v1, W2, g2, b2, m2, v2,
           W3, g3, b3, m3, v3, **_unused):
    args = (W1, g1, b1, m1, v1, W2, g2, b2, m2, v2, W3, g3, b3, m3, v3)
    pos = np.asarray(pos, np.float32)
    x = np.asarray(x, np.float32)
    try:
        import jax

        devs = jax.devices()
        nd = min(len(devs), pos.shape[0])
        per_cloud = _make_device_fn()
        # one point cloud per core; weights replicated (in_axes=None)
        pm = jax.pmap(per_cloud,
                      in_axes=(0, 0) + (None,) * len(args),
                      devices=devs[:nd])
        shards = []
        for s in range(0, pos.shape[0], nd):
            pe = min(s + nd, pos.shape[0])
            out = pm(pos[s:pe], x[s:pe], *args)
            shards.append(np.asarray(out, np.float32))
        y = np.concatenate(shards, axis=0)
        return pos, y
    except Exception:
        return _kernel_numpy(pos, x, *args)


if __name__ == "__main__":
    rng = np.random.default_rng(0)
    d = dict(
        pos=rng.random((B, N, 3), dtype=np.float32),
        x=(rng.standard_normal((B, N, C)) * 0.5).astype(np.float32),
        W1=(rng.standard_normal((C, C + 3)) * 0.05).astype(np.float32),
        g1=rng.random(C).astype(np.float32) + 0.5,
        b1=(rng.standard_normal(C) * 0.1).astype(np.float32),
        m1=(rng.standard_normal(C) * 0.1).astype(np.float32),
        v1=rng.random(C).astype(np.float32) + 0.5,
        W2=(rng.standard_normal((4 * C, C)) * 0.05).astype(np.float32),
        g2=rng.random(4 * C).astype(np.float32) + 0.5,
        b2=(rng.standard_normal(4 * C) * 0.1).astype(np.float32),
        m2=(rng.standard_normal(4 * C) * 0.1).astype(np.float32),
        v2=rng.random(4 * C).astype(np.float32) + 0.5,
        W3=(rng.standard_normal((C, 4 * C)) * 0.05).astype(np.float32),
        g3=rng.random(C).astype(np.float32) + 0.5,
        b3=(rng.standard_normal(C) * 0.1).astype(np.float32),
        m3=(rng.standard_normal(C) * 0.1).astype(np.float32),
        v3=rng.random(C).astype(np.float32) + 0.5,
    )
    p, y = kernel(**d)
    print("out", y.shape, y.dtype, float(np.abs(y).sum()))
